# revision 10
# baseline (speedup 1.0000x reference)
"""KAN layer (spline order k=3, grid G=5, uniform knots) on 8 TRN2 NeuronCores.

Math: the reference's per-unit B-spline basis uses the SAME uniform knot
vector (step h=0.4 on [-2.2, 2.2]) for every (out,in) unit, so the 8 cubic
basis functions are translates of the cardinal cubic B-spline N3:

    B_c(t) = N3(s - c),  s = (t + 2.2)/0.4 = 2.5 t + 5.5
    N3(u)  = z^3/6 - (2/3) relu(z-1)^3,   z = relu(2 - |u - 2|)

(exact, cancellation-free, and reproduces the half-open-interval zero
outside the knot span).  With v = |u-2|: z = relu(2-v), y = relu(1-v)
(y == relu(z-1), but computable straight from v).  The layer is then

    out[b,o] = bias[o] + sum_i W[o,i] silu(x[b,i])
             + sum_{i,c} C6[o,i,c] * phi[b,i,c]

with W = (mask*scale_base), C6 = (mask*scale_spline/6)*coeff and
phi = 6*N3 = z^3 - 4*y^3.  Both contractions are matmuls over i (K=128)
on the tensor engine with PSUM accumulation.

Sharding: pure data-parallel over batch (512 -> 64 per core); weights are
replicated, no collectives.  Host work is layout-only (transpose/reshape);
all arithmetic (including mask/scale folding) happens on-device.
"""

from contextlib import ExitStack

import numpy as np

import concourse.bacc as bacc
import concourse.bass as bass
import concourse.mybir as mybir
import concourse.tile as tile
from concourse.bass_utils import run_bass_kernel_spmd

B, IN, OUT, NCORE = 512, 128, 128, 8
BL = B // NCORE  # 64 batch rows per core
GK = 8           # G + K basis functions per unit
HC = GK // 2     # half of the c-range, for pipelining
F32 = mybir.dt.float32
F32R = mybir.dt.float32r
AF = mybir.ActivationFunctionType
OP = mybir.AluOpType

USE_POW = False   # DVE pow for cubes fails walrus ISA check on TRN2
USE_F32R = False  # fp32r halves PE time but costs 500x accuracy; keep f32

_CACHE = {}


def _build_nc():
    nc = bacc.Bacc(
        "TRN2",
        target_bir_lowering=False,
        debug=False,
        enable_asserts=False,
        num_devices=NCORE,
    )
    xt_d = nc.dram_tensor("xt", [IN, BL], F32, kind="ExternalInput").ap()
    ct_d = nc.dram_tensor("coefft", [IN, GK, OUT], F32, kind="ExternalInput").ap()
    # mask / scale_base / scale_spline concatenated: [IN, 3, OUT]
    sc_d = nc.dram_tensor("scales3", [IN, 3, OUT], F32, kind="ExternalInput").ap()
    bi_d = nc.dram_tensor("biasp", [OUT, 1], F32, kind="ExternalInput").ap()
    out_d = nc.dram_tensor("outt", [OUT, BL], F32, kind="ExternalOutput").ap()

    MMT = F32R if USE_F32R else F32  # matmul-operand tiles: walrus requires
    # fp32r-matmul inputs to be *written* as fp32r by their producers

    with tile.TileContext(nc) as tc, ExitStack() as ctx:
        pool = ctx.enter_context(tc.tile_pool(name="main", bufs=1))
        psum = ctx.enter_context(
            tc.tile_pool(name="psum", bufs=1, space=bass.MemorySpace.PSUM)
        )

        # ---- loads, spread across engines so issue costs overlap ----
        xt = pool.tile([IN, BL], F32)
        nc.sync.dma_start(xt[:], xt_d)
        cw = pool.tile([IN, GK, OUT], F32)
        nc.gpsimd.dma_start(cw[:, 0:4, :], ct_d[:, 0:4, :])
        bi = pool.tile([OUT, 1], F32)
        nc.scalar.dma_start(bi[:], bi_d)
        sc = pool.tile([IN, 3, OUT], F32)
        nc.scalar.dma_start(sc[:], sc_d)
        nc.scalar.dma_start(cw[:, 4:8, :], ct_d[:, 4:8, :])
        mk, sbt, sst = sc[:, 0, :], sc[:, 1, :], sc[:, 2, :]

        # const per-partition bias columns for ACT ops
        cb0 = pool.tile([128, 1], F32)
        nc.gpsimd.memset(cb0[:], 0.0)
        cbm2 = pool.tile([128, 1], F32)
        nc.gpsimd.memset(cbm2[:], -2.0)
        C6 = 6.0 ** (1.0 / 3.0)  # z'=z/C6, y'=y/C6 so phi = z'^3-4y'^3 = N3
        cb2 = pool.tile([128, 1], F32)
        nc.gpsimd.memset(cb2[:], 2.0 / C6)
        cb1 = pool.tile([128, 1], F32)
        nc.gpsimd.memset(cb1[:], 1.0 / C6)

        # ---- 2*silu(x) = x + x*tanh(x/2); the 0.5 is folded into msb.
        # Tanh shares one act table with Abs/Relu/Square/Identity, so the
        # scalar engine loads a single table (sigmoid needed a second). ----
        th = pool.tile([IN, BL], F32)
        nc.scalar.activation(th[:], xt[:], AF.Tanh, bias=cb0[:], scale=0.5)
        xh = pool.tile([IN, BL], F32)
        nc.vector.tensor_scalar(xh[:], xt[:], 0.5, None, OP.mult)
        sx = pool.tile([IN, BL], MMT)
        nc.vector.scalar_tensor_tensor(sx[:], th[:], 1.0, xh[:], OP.add, OP.mult)

        # ---- basis, in two c-halves for pipelining ----
        u = pool.tile([IN, GK, BL], F32)
        for c in range(GK):
            nc.vector.tensor_scalar(
                u[:, c, :], xt[:], 2.5, 5.5 - float(c), OP.mult, OP.add
            )
        v = pool.tile([IN, GK, BL], F32)
        z = pool.tile([IN, GK, BL], F32)
        y = pool.tile([IN, GK, BL], F32)
        z3 = pool.tile([IN, GK, BL], F32)
        y3 = pool.tile([IN, GK, BL], F32)
        phi = pool.tile([IN, GK, BL], MMT)
        if not USE_POW:
            z2 = pool.tile([IN, GK, BL], F32)
            y2 = pool.tile([IN, GK, BL], F32)
        for h in range(2):
            hs = slice(h * HC, (h + 1) * HC)
            nc.scalar.activation(v[:, hs, :], u[:, hs, :], AF.Abs, bias=cbm2[:], scale=1.0)
            nc.scalar.activation(z[:, hs, :], v[:, hs, :], AF.Relu, bias=cb2[:], scale=-1.0 / C6)
            nc.scalar.activation(y[:, hs, :], v[:, hs, :], AF.Relu, bias=cb1[:], scale=-1.0 / C6)
            if USE_POW:
                nc.vector.tensor_scalar(z3[:, hs, :], z[:, hs, :], 3.0, None, OP.pow)
                nc.vector.tensor_scalar(y3[:, hs, :], y[:, hs, :], 3.0, None, OP.pow)
            else:
                nc.scalar.activation(z2[:, hs, :], z[:, hs, :], AF.Square, bias=cb0[:], scale=1.0)
                nc.scalar.activation(y2[:, hs, :], y[:, hs, :], AF.Square, bias=cb0[:], scale=1.0)
                nc.vector.tensor_mul(z3[:, hs, :], z2[:, hs, :], z[:, hs, :])
                nc.vector.tensor_mul(y3[:, hs, :], y2[:, hs, :], y[:, hs, :])
            # phi = (y3 * -4) + z3
            nc.vector.scalar_tensor_tensor(
                phi[:, hs, :], y3[:, hs, :], -4.0, z3[:, hs, :], OP.mult, OP.add
            )

        # ---- fold mask/scales into weights ----
        mss = pool.tile([IN, OUT], F32)
        nc.gpsimd.tensor_mul(mss[:], sst[:], mk[:])
        msb = pool.tile([IN, OUT], MMT)
        nc.gpsimd.tensor_mul(msb[:], sbt[:], mk[:])
        cwm = pool.tile([IN, GK, OUT], MMT)
        for c in range(GK):
            nc.gpsimd.tensor_mul(cwm[:, c, :], cw[:, c, :], mss[:])

        # ---- matmuls: silu term + 8 spline terms accumulate in PSUM ----
        ps = psum.tile([OUT, BL], F32)
        nc.tensor.matmul(ps[:], msb[:], sx[:], start=True, stop=False)
        for c in range(GK):
            nc.tensor.matmul(
                ps[:],
                cwm[:, c, :],
                phi[:, c, :],
                start=False,
                stop=(c == GK - 1),
            )

        # ---- + bias, PSUM -> SBUF -> DRAM ----
        ob = pool.tile([OUT, BL], F32)
        nc.scalar.activation(ob[:], ps[:], AF.Identity, bias=bi[:], scale=1.0)
        nc.sync.dma_start(out_d, ob[:])

    nc.compile()
    return nc


def _prep_in_maps(x, coeff, mask, scale_base, scale_spline, bias):
    f32 = np.float32
    x = np.asarray(x, f32)
    coeff = np.asarray(coeff, f32)
    mask = np.asarray(mask, f32)
    scale_base = np.asarray(scale_base, f32)
    scale_spline = np.asarray(scale_spline, f32)
    bias = np.asarray(bias, f32)

    xT = np.ascontiguousarray(x.T)  # [IN, B]
    # coeff[s, c], s = o*IN + i  ->  [i, c, o]
    coefft = np.ascontiguousarray(coeff.reshape(OUT, IN, GK).transpose(1, 2, 0))
    scales3 = np.ascontiguousarray(
        np.stack(
            [
                mask.reshape(OUT, IN).T,
                scale_base.reshape(OUT, IN).T,
                scale_spline.reshape(OUT, IN).T,
            ],
            axis=1,
        )
    )  # [IN, 3, OUT]
    biasp = np.ascontiguousarray(bias.reshape(OUT, 1))

    in_maps = []
    for j in range(NCORE):
        in_maps.append(
            {
                "xt": np.ascontiguousarray(xT[:, j * BL : (j + 1) * BL]),
                "coefft": coefft,
                "scales3": scales3,
                "biasp": biasp,
            }
        )
    return in_maps


def run(x, coeff, mask, scale_base, scale_spline, bias, trace=False):
    if "nc" not in _CACHE:
        _CACHE["nc"] = _build_nc()
    nc = _CACHE["nc"]
    in_maps = _prep_in_maps(x, coeff, mask, scale_base, scale_spline, bias)
    res = run_bass_kernel_spmd(
        nc, in_maps, core_ids=list(range(NCORE)), trace=trace
    )
    outT = np.concatenate(
        [res.results[j]["outt"] for j in range(NCORE)], axis=1
    )  # [OUT, B]
    return np.ascontiguousarray(outT.T), res


def kernel(x, grid, coeff, mask, scale_base, scale_spline, bias, k):
    assert int(np.asarray(k)) == 3
    out, _ = run(x, coeff, mask, scale_base, scale_spline, bias, trace=False)
    return out
